# revision 3
# baseline (speedup 1.0000x reference)
"""RoPE + ALiBi single-head attention (B=8, T=2048, H=256) on 8 Trainium2
cores, batch-parallel (one batch element per core).

Per-core algorithm (all compute on device):
  qeT/keT = RoPE(qT/kT)                     [DVE, fp32 -> fp32r]
  scoresT[s,t] = sum_d keT[d,s]*qeT[d,t]    [PE, fp32r, 2 k-tiles]
  at[s,t] = exp(scoresT*scale + slope*s)    [ACT, PSUM->SBUF fp32r]
     (the -slope*t alibi term is constant per softmax column and cancels)
  den[t] = sum_s at[s,t]                    [DVE accumulate + GpSimd
                                             partition_all_reduce]
  outT[h,t] = (sum_s v[s,h]*at[s,t]) / den  [PE fp32r + DVE normalize]
Host only reshapes/transposes and precomputes the rope/alibi tables.
"""
import math
from contextlib import ExitStack

import numpy as np

import concourse.bacc as bacc
import concourse.bass_isa as bass_isa
import concourse.tile as tile
from concourse import mybir
from concourse.bass_utils import run_bass_kernel_spmd

B, T, H = 8, 2048, 256
HALF = H // 2          # 128 (rope half, also partition dim)
NCHUNK = 4
CHUNK = T // NCHUNK    # 512 query columns per chunk
NS = T // 128          # 16 key tiles
ROPE_BASE = 10000.0
SLOPE = 2.0 ** (-8.0)
SCALE = 1.0 / math.sqrt(H)

F32 = mybir.dt.float32
F32R = mybir.dt.float32r
EXP = mybir.ActivationFunctionType.Exp
MULT = mybir.AluOpType.mult
ADD = mybir.AluOpType.add

TRACE = False           # test harness sets True for NTFF profiling
LAST_RESULTS = None     # BassKernelResults of the last run (for profiling)

_NC_CACHE = {}


def _build_nc():
    nc = bacc.Bacc("TRN2", target_bir_lowering=False, debug=False)
    qt_d = nc.dram_tensor("qt", [H, T], F32, kind="ExternalInput").ap()
    kt_d = nc.dram_tensor("kt", [H, T], F32, kind="ExternalInput").ap()
    v_d = nc.dram_tensor("v", [T, H], F32, kind="ExternalInput").ap()
    cos_d = nc.dram_tensor("costab", [HALF, T], F32, kind="ExternalInput").ap()
    sin_d = nc.dram_tensor("sintab", [HALF, T], F32, kind="ExternalInput").ap()
    bias_d = nc.dram_tensor("alibi", [128, NS], F32, kind="ExternalInput").ap()
    ot_d = nc.dram_tensor("ot", [H, T], F32, kind="ExternalOutput").ap()

    with tile.TileContext(nc) as tc, ExitStack() as ctx:
        const = ctx.enter_context(tc.tile_pool(name="const", bufs=1))
        rpool = ctx.enter_context(tc.tile_pool(name="ropeout", bufs=1))
        vpool = ctx.enter_context(tc.tile_pool(name="vpool", bufs=1))

        biasb = const.tile([128, NS], F32)
        nc.sync.dma_start(biasb[:], bias_d[:])

        # persistent fp32r operands for the two GEMMs
        qe = [rpool.tile([128, T], F32R, name=f"qe{i}", tag=f"qe{i}")
              for i in range(2)]
        ke = [rpool.tile([128, T], F32R, name=f"ke{i}", tag=f"ke{i}")
              for i in range(2)]
        vr = vpool.tile([128, NS * H], F32R)

        with tc.tile_pool(name="stage", bufs=1) as stage:
            cosb = stage.tile([128, T], F32, tag="cosb")
            nc.sync.dma_start(cosb[:], cos_d[:])
            sinb = stage.tile([128, T], F32, tag="sinb")
            nc.sync.dma_start(sinb[:], sin_d[:])
            for name, src, dst in (("k", kt_d, ke), ("q", qt_d, qe)):
                s0 = stage.tile([128, T], F32, tag="s0", bufs=2, name=f"{name}s0")
                nc.sync.dma_start(s0[:], src[0:128, :])
                s1 = stage.tile([128, T], F32, tag="s1", bufs=2, name=f"{name}s1")
                nc.sync.dma_start(s1[:], src[128:256, :])
                # dst0 = s0*cos - s1*sin ; dst1 = s1*cos + s0*sin
                nc.vector.tensor_mul(dst[0][:], s0[:], cosb[:])
                tmp = stage.tile([128, T], F32, tag="tmp", bufs=2, name=f"{name}tmp0")
                nc.vector.tensor_mul(tmp[:], s1[:], sinb[:])
                nc.vector.tensor_sub(dst[0][:], dst[0][:], tmp[:])
                nc.vector.tensor_mul(dst[1][:], s1[:], cosb[:])
                tmp2 = stage.tile([128, T], F32, tag="tmp", bufs=2, name=f"{name}tmp1")
                nc.vector.tensor_mul(tmp2[:], s0[:], sinb[:])
                nc.vector.tensor_add(dst[1][:], dst[1][:], tmp2[:])
            for s in range(NS):
                vst = stage.tile([128, H], F32, tag="vst", bufs=3, name=f"vst{s}")
                nc.sync.dma_start(vst[:], v_d[s * 128:(s + 1) * 128, :])
                nc.vector.tensor_copy(vr[:, s * H:(s + 1) * H], vst[:])

        atp = ctx.enter_context(tc.tile_pool(name="atp", bufs=32))
        sp = ctx.enter_context(tc.tile_pool(name="sp", bufs=2))
        dn = ctx.enter_context(tc.tile_pool(name="dn", bufs=2))
        onp = ctx.enter_context(tc.tile_pool(name="onp", bufs=4))
        ps1p = ctx.enter_context(tc.tile_pool(name="ps1", bufs=3, space="PSUM"))
        ps2p = ctx.enter_context(tc.tile_pool(name="ps2", bufs=3, space="PSUM"))

        mm = nc.tensor.matmul
        for c in range(NCHUNK):
            tcol = slice(c * CHUNK, (c + 1) * CHUNK)
            at_tiles = []
            S = sp.tile([128, CHUNK], F32)
            for s in range(NS):
                p1 = ps1p.tile([128, CHUNK], F32)
                mm(p1[:], ke[0][:, s * 128:(s + 1) * 128], qe[0][:, tcol],
                   start=True, stop=False)
                mm(p1[:], ke[1][:, s * 128:(s + 1) * 128], qe[1][:, tcol],
                   start=False, stop=True)
                at = atp.tile([128, CHUNK], F32R, tag="at")
                nc.scalar.activation(at[:], p1[:], EXP,
                                     bias=biasb[:, s:s + 1], scale=SCALE)
                if s == 0:
                    nc.vector.tensor_copy(S[:], at[:])
                else:
                    nc.vector.tensor_add(S[:], S[:], at[:])
                at_tiles.append(at)

            denb = dn.tile([128, CHUNK], F32, tag="denb")
            nc.gpsimd.partition_all_reduce(denb[:], S[:], 128,
                                           bass_isa.ReduceOp.add)
            r0 = dn.tile([128, CHUNK], F32, tag="r0")
            nc.vector.reciprocal(r0[:], denb[:])
            # one Newton step: recip = r0 * (2 - den*r0)
            t2 = dn.tile([128, CHUNK], F32, tag="t2")
            nc.vector.scalar_tensor_tensor(t2[:], denb[:], -1.0, r0[:], MULT, MULT)
            recipb = dn.tile([128, CHUNK], F32, tag="recipb")
            nc.vector.scalar_tensor_tensor(recipb[:], t2[:], 2.0, r0[:], ADD, MULT)

            for h in range(2):
                p2 = ps2p.tile([128, CHUNK], F32)
                for s in range(NS):
                    mm(p2[:], vr[:, s * H + h * 128: s * H + h * 128 + 128],
                       at_tiles[s][:], start=(s == 0), stop=(s == NS - 1))
                on = onp.tile([128, CHUNK], F32)
                nc.vector.tensor_mul(on[:], p2[:], recipb[:])
                nc.sync.dma_start(ot_d[h * 128:(h + 1) * 128, tcol], on[:])

    nc.compile()
    return nc


def _get_nc():
    if "nc" not in _NC_CACHE:
        _NC_CACHE["nc"] = _build_nc()
    return _NC_CACHE["nc"]


def _tables():
    j = np.arange(HALF, dtype=np.float64)
    inv = ROPE_BASE ** (-2.0 * j / H)
    t = np.arange(T, dtype=np.float64)
    fr = np.outer(inv, t)                       # [128, T]
    cos = np.cos(fr).astype(np.float32)
    sin = np.sin(fr).astype(np.float32)
    p = np.arange(128, dtype=np.float64)[:, None]
    sidx = p + 128.0 * np.arange(NS, dtype=np.float64)[None, :]
    bias = (SLOPE * sidx).astype(np.float32)    # [128, NS]
    return cos, sin, bias


def kernel(q, k, v):
    global LAST_RESULTS
    q = np.asarray(q, dtype=np.float32)
    k = np.asarray(k, dtype=np.float32)
    v = np.asarray(v, dtype=np.float32)
    assert q.shape == (B, T, H), q.shape

    nc = _get_nc()
    cos, sin, bias = _tables()
    in_maps = []
    for b in range(B):
        in_maps.append({
            "qt": np.ascontiguousarray(q[b].T),
            "kt": np.ascontiguousarray(k[b].T),
            "v": np.ascontiguousarray(v[b]),
            "costab": cos,
            "sintab": sin,
            "alibi": bias,
        })
    kw = {}
    if TRACE:
        kw = dict(trace=True)
    res = run_bass_kernel_spmd(nc, in_maps, list(range(B)), **kw)
    LAST_RESULTS = res
    out = np.stack(
        [np.ascontiguousarray(res.results[b]["ot"]).T for b in range(B)], axis=0
    )
    return out[None].astype(np.float32)


# revision 4
# speedup vs baseline: 1.1315x; 1.1315x over previous
"""RoPE + ALiBi single-head attention (B=8, T=2048, H=256) on 8 Trainium2
cores, batch-parallel (one batch element per core).

Per-core algorithm (all compute on device):
  qeT/keT = RoPE(qT/kT)                     [DVE, fp32 -> fp32r]
  scoresT[s,t] = sum_d keT[d,s]*qeT[d,t]    [PE, fp32r, 2 k-tiles]
  at[s,t] = exp(scoresT*scale + slope*s)    [ACT, PSUM->SBUF fp32r]
     (the -slope*t alibi term is constant per softmax column and cancels)
  den[t] = sum_s at[s,t]                    [DVE accumulate (fp32r) + one
                                             ones-matmul partition reduce]
  outT[h,t] = (sum_s v[s,h]*at[s,t]) / den  [PE fp32r + DVE normalize,
                                             recip broadcast via GpSimd]
Host only reshapes/transposes and precomputes the rope/alibi tables.
"""
import math
from contextlib import ExitStack

import numpy as np

import concourse.bacc as bacc
import concourse.tile as tile
from concourse import mybir
from concourse.bass_utils import run_bass_kernel_spmd

B, T, H = 8, 2048, 256
HALF = H // 2          # 128 (rope half, also partition dim)
NCHUNK = 4
CHUNK = T // NCHUNK    # 512 query columns per chunk
NS = T // 128          # 16 key tiles
ROPE_BASE = 10000.0
SLOPE = 2.0 ** (-8.0)
SCALE = 1.0 / math.sqrt(H)

F32 = mybir.dt.float32
F32R = mybir.dt.float32r
EXP = mybir.ActivationFunctionType.Exp
MULT = mybir.AluOpType.mult
ADD = mybir.AluOpType.add

TRACE = False           # test harness sets True for NTFF profiling
LAST_RESULTS = None     # BassKernelResults of the last run (for profiling)

_NC_CACHE = {}


def _build_nc():
    nc = bacc.Bacc("TRN2", target_bir_lowering=False, debug=False)
    qt_d = nc.dram_tensor("qt", [H, T], F32, kind="ExternalInput").ap()
    kt_d = nc.dram_tensor("kt", [H, T], F32, kind="ExternalInput").ap()
    v_d = nc.dram_tensor("v", [T, H], F32, kind="ExternalInput").ap()
    cos_d = nc.dram_tensor("costab", [HALF, T], F32, kind="ExternalInput").ap()
    sin_d = nc.dram_tensor("sintab", [HALF, T], F32, kind="ExternalInput").ap()
    bias_d = nc.dram_tensor("alibi", [128, NS], F32, kind="ExternalInput").ap()
    ot_d = nc.dram_tensor("ot", [H, T], F32, kind="ExternalOutput").ap()

    with tile.TileContext(nc) as tc, ExitStack() as ctx:
        const = ctx.enter_context(tc.tile_pool(name="const", bufs=1))
        rpool = ctx.enter_context(tc.tile_pool(name="ropeout", bufs=1))
        vpool = ctx.enter_context(tc.tile_pool(name="vpool", bufs=1))
        stage = ctx.enter_context(tc.tile_pool(name="stage", bufs=1))
        atp = ctx.enter_context(tc.tile_pool(name="atp", bufs=24))
        sp = ctx.enter_context(tc.tile_pool(name="sp", bufs=2))
        dn = ctx.enter_context(tc.tile_pool(name="dn", bufs=2))
        onp = ctx.enter_context(tc.tile_pool(name="onp", bufs=4))
        ps1p = ctx.enter_context(tc.tile_pool(name="ps1", bufs=3, space="PSUM"))
        ps2p = ctx.enter_context(tc.tile_pool(name="ps2", bufs=3, space="PSUM"))
        pdnp = ctx.enter_context(tc.tile_pool(name="pdn", bufs=2, space="PSUM"))

        # small constants: alibi bias (gpsimd queue), ones column for the
        # denominator partition-reduce matmul
        biasb = const.tile([128, NS], F32)
        nc.gpsimd.dma_start(biasb[:], bias_d[:])
        ones_f = const.tile([128, 1], F32)
        nc.vector.memset(ones_f[:], 1.0)
        ones_r = const.tile([128, 1], F32R)
        nc.vector.tensor_copy(ones_r[:], ones_f[:])

        # persistent fp32r operands for the two GEMMs
        qe = [rpool.tile([128, T], F32R, name=f"qe{i}", tag=f"qe{i}")
              for i in range(2)]
        ke = [rpool.tile([128, T], F32R, name=f"ke{i}", tag=f"ke{i}")
              for i in range(2)]
        vr = vpool.tile([128, NS * H], F32R)

        # rope tables + k/q staging (sync HWDGE queues)
        cosb = stage.tile([128, T], F32, tag="cosb")
        nc.sync.dma_start(cosb[:], cos_d[:])
        sinb = stage.tile([128, T], F32, tag="sinb")
        nc.sync.dma_start(sinb[:], sin_d[:])

        def rope(src0, src1, dst, col, tmptag):
            """dst0[:,col] = s0*cos - s1*sin ; dst1[:,col] = s1*cos + s0*sin"""
            nc.vector.tensor_mul(dst[0][:, col], src0[:, col], cosb[:, col])
            tmp = stage.tile([128, col.stop - col.start], F32, tag=tmptag,
                             bufs=2, name=f"tmp{tmptag}{col.start}")
            nc.vector.tensor_mul(tmp[:], src1[:, col], sinb[:, col])
            nc.vector.tensor_sub(dst[0][:, col], dst[0][:, col], tmp[:])
            nc.vector.tensor_mul(dst[1][:, col], src1[:, col], cosb[:, col])
            tmp2 = stage.tile([128, col.stop - col.start], F32, tag=tmptag,
                              bufs=2, name=f"tmp2{tmptag}{col.start}")
            nc.vector.tensor_mul(tmp2[:], src0[:, col], sinb[:, col])
            nc.vector.tensor_add(dst[1][:, col], dst[1][:, col], tmp2[:])

        ks0 = stage.tile([128, T], F32, tag="ks0")
        nc.sync.dma_start(ks0[:], kt_d[0:128, :])
        ks1 = stage.tile([128, T], F32, tag="ks1")
        nc.sync.dma_start(ks1[:], kt_d[128:256, :])
        qs0 = stage.tile([128, T], F32, tag="qs0")
        nc.sync.dma_start(qs0[:], qt_d[0:128, :])
        qs1 = stage.tile([128, T], F32, tag="qs1")
        nc.sync.dma_start(qs1[:], qt_d[128:256, :])

        # k rope, full width (GEMM1 needs every key column)
        rope(ks0, ks1, ke, slice(0, T), "ktmp")
        # q rope for chunk 0 only; later chunks are roped one chunk ahead
        rope(qs0, qs1, qe, slice(0, CHUNK), "qtmp")

        # v load + fp32r cast entirely on gpsimd (own DMA queues, own ALU)
        for s in range(NS):
            vst = stage.tile([128, H], F32, tag="vst", bufs=4, name=f"vst{s}")
            nc.gpsimd.dma_start(vst[:], v_d[s * 128:(s + 1) * 128, :])
            nc.gpsimd.tensor_copy(vr[:, s * H:(s + 1) * H], vst[:])

        mm = nc.tensor.matmul
        for c in range(NCHUNK):
            tcol = slice(c * CHUNK, (c + 1) * CHUNK)
            if c + 1 < NCHUNK:
                # rope next chunk's q columns ahead of its GEMM1
                rope(qs0, qs1, qe, slice((c + 1) * CHUNK, (c + 2) * CHUNK),
                     "qtmp")
            at_tiles = []
            S = sp.tile([128, CHUNK], F32R)
            for s in range(NS):
                p1 = ps1p.tile([128, CHUNK], F32)
                mm(p1[:], ke[0][:, s * 128:(s + 1) * 128], qe[0][:, tcol],
                   start=True, stop=False)
                mm(p1[:], ke[1][:, s * 128:(s + 1) * 128], qe[1][:, tcol],
                   start=False, stop=True)
                at = atp.tile([128, CHUNK], F32R, tag="at")
                nc.scalar.activation(at[:], p1[:], EXP,
                                     bias=biasb[:, s:s + 1], scale=SCALE)
                if s == 0:
                    nc.vector.tensor_copy(S[:], at[:])
                else:
                    nc.vector.tensor_add(S[:], S[:], at[:])
                at_tiles.append(at)

            # denominator: ones.T @ S -> [1, CHUNK]; reciprocal + one Newton
            # step on the single row; broadcast to 128 partitions on gpsimd
            pden = pdnp.tile([1, CHUNK], F32)
            mm(pden[:], ones_r[:, 0:1], S[:], start=True, stop=True)
            r0 = dn.tile([1, CHUNK], F32, tag="r0")
            nc.vector.reciprocal(r0[:], pden[0:1, :])
            t2 = dn.tile([1, CHUNK], F32, tag="t2")
            nc.vector.scalar_tensor_tensor(t2[:], pden[0:1, :], -1.0, r0[:],
                                           MULT, MULT)
            r1 = dn.tile([1, CHUNK], F32, tag="r1")
            nc.vector.scalar_tensor_tensor(r1[:], t2[:], 2.0, r0[:], ADD, MULT)
            recipb = dn.tile([128, CHUNK], F32, tag="recipb")
            nc.gpsimd.partition_broadcast(recipb[:], r1[0:1, :], 128)

            for h in range(2):
                p2 = ps2p.tile([128, CHUNK], F32)
                for s in range(NS):
                    mm(p2[:], vr[:, s * H + h * 128: s * H + h * 128 + 128],
                       at_tiles[s][:], start=(s == 0), stop=(s == NS - 1))
                on = onp.tile([128, CHUNK], F32)
                nc.vector.tensor_mul(on[:], p2[:], recipb[:])
                nc.sync.dma_start(ot_d[h * 128:(h + 1) * 128, tcol], on[:])

    nc.compile()
    return nc


def _get_nc():
    if "nc" not in _NC_CACHE:
        _NC_CACHE["nc"] = _build_nc()
    return _NC_CACHE["nc"]


def _tables():
    j = np.arange(HALF, dtype=np.float64)
    inv = ROPE_BASE ** (-2.0 * j / H)
    t = np.arange(T, dtype=np.float64)
    fr = np.outer(inv, t)                       # [128, T]
    cos = np.cos(fr).astype(np.float32)
    sin = np.sin(fr).astype(np.float32)
    p = np.arange(128, dtype=np.float64)[:, None]
    sidx = p + 128.0 * np.arange(NS, dtype=np.float64)[None, :]
    bias = (SLOPE * sidx).astype(np.float32)    # [128, NS]
    return cos, sin, bias


def kernel(q, k, v):
    global LAST_RESULTS
    q = np.asarray(q, dtype=np.float32)
    k = np.asarray(k, dtype=np.float32)
    v = np.asarray(v, dtype=np.float32)
    assert q.shape == (B, T, H), q.shape

    nc = _get_nc()
    cos, sin, bias = _tables()
    in_maps = []
    for b in range(B):
        in_maps.append({
            "qt": np.ascontiguousarray(q[b].T),
            "kt": np.ascontiguousarray(k[b].T),
            "v": np.ascontiguousarray(v[b]),
            "costab": cos,
            "sintab": sin,
            "alibi": bias,
        })
    kw = {}
    if TRACE:
        kw = dict(trace=True)
    res = run_bass_kernel_spmd(nc, in_maps, list(range(B)), **kw)
    LAST_RESULTS = res
    out = np.stack(
        [np.ascontiguousarray(res.results[b]["ot"]).T for b in range(B)], axis=0
    )
    return out[None].astype(np.float32)


# revision 5
# speedup vs baseline: 1.3244x; 1.1705x over previous
"""RoPE + ALiBi single-head attention (B=8, T=2048, H=256) on 8 Trainium2
cores, batch-parallel (one batch element per core).

Per-core algorithm (all compute on device):
  qeT/keT = RoPE(qT/kT)                     [DVE, fp32 -> fp32r, pipelined
                                             with the input DMA in 512-col
                                             chunks so GEMM1 starts early]
  scoresT[s,t] = sum_d keT[d,s]*qeT[d,t]    [PE, fp32r, 2 k-tiles]
  at[s,t] = exp(scoresT*scale + slope*s)    [ACT, PSUM->SBUF fp32r]
     (the -slope*t alibi term is constant per softmax column and cancels)
  den[t] = sum_s at[s,t]                    [PE: 16 accumulating ones-matmuls
                                             into a [1,512] PSUM row]
  outT[h,t] = (sum_s v[s,h]*at[s,t]) / den  [PE fp32r; reciprocal via magic
                                             bit-trick + 3 Newton steps on
                                             the [1,512] row, broadcast on
                                             GpSimd, DVE normalize]
Host only reshapes/transposes and precomputes the rope/alibi tables.
"""
import math
from contextlib import ExitStack

import numpy as np

import concourse.bacc as bacc
import concourse.tile as tile
from concourse import mybir
from concourse.bass_utils import run_bass_kernel_spmd

B, T, H = 8, 2048, 256
HALF = H // 2          # 128 (rope half, also partition dim)
NCHUNK = 4
CHUNK = T // NCHUNK    # 512 query columns per chunk
NS = T // 128          # 16 key tiles
ROPE_BASE = 10000.0
SLOPE = 2.0 ** (-8.0)
SCALE = 1.0 / math.sqrt(H)
RECIP_MAGIC = 0x7EF127EA  # fast fp32 reciprocal seed: magic - bits(x)

F32 = mybir.dt.float32
F32R = mybir.dt.float32r
I32 = mybir.dt.int32
EXP = mybir.ActivationFunctionType.Exp
MULT = mybir.AluOpType.mult
ADD = mybir.AluOpType.add

TRACE = False           # test harness sets True for NTFF profiling
LAST_RESULTS = None     # BassKernelResults of the last run (for profiling)

_NC_CACHE = {}


def _build_nc():
    nc = bacc.Bacc("TRN2", target_bir_lowering=False, debug=False)
    qt_d = nc.dram_tensor("qt", [H, T], F32, kind="ExternalInput").ap()
    kt_d = nc.dram_tensor("kt", [H, T], F32, kind="ExternalInput").ap()
    v_d = nc.dram_tensor("v", [T, H], F32, kind="ExternalInput").ap()
    cos_d = nc.dram_tensor("costab", [HALF, T], F32, kind="ExternalInput").ap()
    sin_d = nc.dram_tensor("sintab", [HALF, T], F32, kind="ExternalInput").ap()
    bias_d = nc.dram_tensor("alibi", [128, NS], F32, kind="ExternalInput").ap()
    ot_d = nc.dram_tensor("ot", [H, T], F32, kind="ExternalOutput").ap()

    with tile.TileContext(nc) as tc, ExitStack() as ctx:
        const = ctx.enter_context(tc.tile_pool(name="const", bufs=1))
        rpool = ctx.enter_context(tc.tile_pool(name="ropeout", bufs=1))
        vpool = ctx.enter_context(tc.tile_pool(name="vpool", bufs=1))
        stage = ctx.enter_context(tc.tile_pool(name="stage", bufs=1))
        atp = ctx.enter_context(tc.tile_pool(name="atp", bufs=24))
        dn = ctx.enter_context(tc.tile_pool(name="dn", bufs=2))
        onp = ctx.enter_context(tc.tile_pool(name="onp", bufs=4))
        ps1p = ctx.enter_context(tc.tile_pool(name="ps1", bufs=3, space="PSUM"))
        ps2p = ctx.enter_context(tc.tile_pool(name="ps2", bufs=3, space="PSUM"))
        pdnp = ctx.enter_context(tc.tile_pool(name="pdn", bufs=2, space="PSUM"))

        # small constants: alibi bias (gpsimd queue), ones column for the
        # denominator partition-reduce matmuls, reciprocal magic row
        biasb = const.tile([128, NS], F32)
        nc.gpsimd.dma_start(biasb[:], bias_d[:])
        ones_f = const.tile([128, 1], F32)
        nc.vector.memset(ones_f[:], 1.0)
        ones_r = const.tile([128, 1], F32R)
        nc.vector.tensor_copy(ones_r[:], ones_f[:])
        magicb = const.tile([1, CHUNK], I32)
        nc.vector.memset(magicb[:], RECIP_MAGIC)

        # persistent fp32r operands for the two GEMMs
        qe = [rpool.tile([128, T], F32R, name=f"qe{i}", tag=f"qe{i}")
              for i in range(2)]
        ke = [rpool.tile([128, T], F32R, name=f"ke{i}", tag=f"ke{i}")
              for i in range(2)]
        vr = vpool.tile([128, NS * H], F32R)

        # full-width staging tiles, filled by per-chunk DMAs (subtile deps
        # let rope/GEMM1 start as soon as their columns land)
        cosb = stage.tile([128, T], F32, tag="cosb")
        sinb = stage.tile([128, T], F32, tag="sinb")
        ks0 = stage.tile([128, T], F32, tag="ks0")
        ks1 = stage.tile([128, T], F32, tag="ks1")
        qs0 = stage.tile([128, T], F32, tag="qs0")
        qs1 = stage.tile([128, T], F32, tag="qs1")

        def load_cols(cc):
            col = slice(cc * CHUNK, (cc + 1) * CHUNK)
            for dst, src in ((cosb, cos_d), (sinb, sin_d),
                             (ks0, kt_d[0:128, :]), (ks1, kt_d[128:256, :])):
                nc.sync.dma_start(dst[:, col], src[:, col])

        def load_q_cols(cc):
            col = slice(cc * CHUNK, (cc + 1) * CHUNK)
            nc.sync.dma_start(qs0[:, col], qt_d[0:128, col])
            nc.sync.dma_start(qs1[:, col], qt_d[128:256, col])

        def rope(src0, src1, dst, col, tmptag):
            """dst0[:,col] = s0*cos - s1*sin ; dst1[:,col] = s1*cos + s0*sin"""
            n = col.stop - col.start
            nc.vector.tensor_mul(dst[0][:, col], src0[:, col], cosb[:, col])
            tmp = stage.tile([128, n], F32, tag="rtmp", bufs=3,
                             name=f"tmp{tmptag}{col.start}")
            nc.vector.tensor_mul(tmp[:], src1[:, col], sinb[:, col])
            nc.vector.tensor_sub(dst[0][:, col], dst[0][:, col], tmp[:])
            nc.vector.tensor_mul(dst[1][:, col], src1[:, col], cosb[:, col])
            tmp2 = stage.tile([128, n], F32, tag="rtmp", bufs=3,
                              name=f"tmp2{tmptag}{col.start}")
            nc.vector.tensor_mul(tmp2[:], src0[:, col], sinb[:, col])
            nc.vector.tensor_add(dst[1][:, col], dst[1][:, col], tmp2[:])

        # chunk-0 inputs first, then k/q rope pipelined with remaining DMAs
        load_cols(0)
        load_q_cols(0)
        rope(ks0, ks1, ke, slice(0, CHUNK), "k0")
        rope(qs0, qs1, qe, slice(0, CHUNK), "q0")
        for cc in range(1, NCHUNK):
            load_cols(cc)
            load_q_cols(cc)
            rope(ks0, ks1, ke, slice(cc * CHUNK, (cc + 1) * CHUNK), f"k{cc}")

        # v load + fp32r cast entirely on gpsimd (own DMA queues, own ALU)
        for s in range(NS):
            vst = stage.tile([128, H], F32, tag="vst", bufs=4, name=f"vst{s}")
            nc.gpsimd.dma_start(vst[:], v_d[s * 128:(s + 1) * 128, :])
            nc.gpsimd.tensor_copy(vr[:, s * H:(s + 1) * H], vst[:])

        mm = nc.tensor.matmul
        for c in range(NCHUNK):
            tcol = slice(c * CHUNK, (c + 1) * CHUNK)
            if c + 1 < NCHUNK:
                # rope next chunk's q columns ahead of its GEMM1
                rope(qs0, qs1, qe, slice((c + 1) * CHUNK, (c + 2) * CHUNK),
                     f"q{c + 1}")
            at_tiles = []
            pden = pdnp.tile([1, CHUNK], F32)
            for s in range(NS):
                p1 = ps1p.tile([128, CHUNK], F32)
                mm(p1[:], ke[0][:, s * 128:(s + 1) * 128], qe[0][:, tcol],
                   start=True, stop=False)
                mm(p1[:], ke[1][:, s * 128:(s + 1) * 128], qe[1][:, tcol],
                   start=False, stop=True)
                if s > 0:
                    # denominator ones-matmul, one tile behind the exps so
                    # the PE never waits on the ACT stream
                    mm(pden[:], ones_r[:, 0:1], at_tiles[s - 1][:],
                       start=(s == 1), stop=False)
                at = atp.tile([128, CHUNK], F32R, tag="at")
                nc.scalar.activation(at[:], p1[:], EXP,
                                     bias=biasb[:, s:s + 1], scale=SCALE)
                at_tiles.append(at)
            mm(pden[:], ones_r[:, 0:1], at_tiles[NS - 1][:],
               start=False, stop=True)

            # reciprocal of the [1, CHUNK] denominator row:
            # seed r = bits(magic - bits(d)), then 3 Newton steps
            den_sb = dn.tile([1, CHUNK], F32, tag="den_sb")
            nc.vector.tensor_copy(den_sb[:], pden[0:1, :])
            r = dn.tile([1, CHUNK], F32, tag="rA", name=f"rA{c}")
            nc.vector.tensor_sub(r[:].bitcast(I32), magicb[:],
                                 den_sb[:].bitcast(I32))
            for it in range(3):
                t2 = dn.tile([1, CHUNK], F32, tag="nt", bufs=2,
                             name=f"nt{c}_{it}")
                nc.vector.scalar_tensor_tensor(t2[:], den_sb[:], -1.0, r[:],
                                               MULT, MULT)
                r_new = dn.tile([1, CHUNK], F32, tag=f"r{it % 2}", bufs=2,
                                name=f"r{c}_{it}")
                nc.vector.scalar_tensor_tensor(r_new[:], t2[:], 2.0, r[:],
                                               ADD, MULT)
                r = r_new
            recipb = dn.tile([128, CHUNK], F32, tag="recipb")
            nc.gpsimd.partition_broadcast(recipb[:], r[0:1, :], 128)

            for h in range(2):
                p2 = ps2p.tile([128, CHUNK], F32)
                for s in range(NS):
                    mm(p2[:], vr[:, s * H + h * 128: s * H + h * 128 + 128],
                       at_tiles[s][:], start=(s == 0), stop=(s == NS - 1))
                on = onp.tile([128, CHUNK], F32)
                nc.vector.tensor_mul(on[:], p2[:], recipb[:])
                nc.sync.dma_start(ot_d[h * 128:(h + 1) * 128, tcol], on[:])

    nc.compile()
    return nc


def _get_nc():
    if "nc" not in _NC_CACHE:
        _NC_CACHE["nc"] = _build_nc()
    return _NC_CACHE["nc"]


def _tables():
    j = np.arange(HALF, dtype=np.float64)
    inv = ROPE_BASE ** (-2.0 * j / H)
    t = np.arange(T, dtype=np.float64)
    fr = np.outer(inv, t)                       # [128, T]
    cos = np.cos(fr).astype(np.float32)
    sin = np.sin(fr).astype(np.float32)
    p = np.arange(128, dtype=np.float64)[:, None]
    sidx = p + 128.0 * np.arange(NS, dtype=np.float64)[None, :]
    bias = (SLOPE * sidx).astype(np.float32)    # [128, NS]
    return cos, sin, bias


def kernel(q, k, v):
    global LAST_RESULTS
    q = np.asarray(q, dtype=np.float32)
    k = np.asarray(k, dtype=np.float32)
    v = np.asarray(v, dtype=np.float32)
    assert q.shape == (B, T, H), q.shape

    nc = _get_nc()
    cos, sin, bias = _tables()
    in_maps = []
    for b in range(B):
        in_maps.append({
            "qt": np.ascontiguousarray(q[b].T),
            "kt": np.ascontiguousarray(k[b].T),
            "v": np.ascontiguousarray(v[b]),
            "costab": cos,
            "sintab": sin,
            "alibi": bias,
        })
    kw = {}
    if TRACE:
        kw = dict(trace=True)
    res = run_bass_kernel_spmd(nc, in_maps, list(range(B)), **kw)
    LAST_RESULTS = res
    out = np.stack(
        [np.ascontiguousarray(res.results[b]["ot"]).T for b in range(B)], axis=0
    )
    return out[None].astype(np.float32)


# revision 6
# speedup vs baseline: 1.3297x; 1.0040x over previous
"""RoPE + ALiBi single-head attention (B=8, T=2048, H=256) on 8 Trainium2
cores, batch-parallel (one batch element per core).

Per-core algorithm (all compute on device):
  qeT/keT = RoPE(qT/kT)                     [DVE, fp32 -> fp32r, pipelined
                                             with the input DMA in 512-col
                                             chunks so GEMM1 starts early]
  scoresT[s,t] = sum_d keT[d,s]*qeT[d,t]    [PE, fp32r, 2 k-tiles]
  at[s,t] = exp(scoresT*scale + slope*s)    [ACT, PSUM->SBUF fp32r]
     (the -slope*t alibi term is constant per softmax column and cancels)
  den[t] = sum_s at[s,t]                    [PE: 16 accumulating ones-matmuls
                                             into a [1,512] PSUM row]
  outT[h,t] = (sum_s v[s,h]*at[s,t]) / den  [PE fp32r; reciprocal via magic
                                             bit-trick + 3 Newton steps on
                                             the [1,512] row, broadcast on
                                             GpSimd, DVE normalize]
Host only reshapes/transposes and precomputes the rope/alibi tables.
"""
import math
from contextlib import ExitStack

import numpy as np

import concourse.bacc as bacc
import concourse.tile as tile
from concourse import mybir
from concourse.bass_utils import run_bass_kernel_spmd

B, T, H = 8, 2048, 256
HALF = H // 2          # 128 (rope half, also partition dim)
NCHUNK = 4
CHUNK = T // NCHUNK    # 512 query columns per chunk
NS = T // 128          # 16 key tiles
ROPE_BASE = 10000.0
SLOPE = 2.0 ** (-8.0)
SCALE = 1.0 / math.sqrt(H)
RECIP_MAGIC = 0x7EF127EA  # fast fp32 reciprocal seed: magic - bits(x)

F32 = mybir.dt.float32
F32R = mybir.dt.float32r
I32 = mybir.dt.int32
EXP = mybir.ActivationFunctionType.Exp
MULT = mybir.AluOpType.mult
ADD = mybir.AluOpType.add

TRACE = False           # test harness sets True for NTFF profiling
LAST_RESULTS = None     # BassKernelResults of the last run (for profiling)

_NC_CACHE = {}


def _build_nc():
    nc = bacc.Bacc("TRN2", target_bir_lowering=False, debug=False)
    qt_d = nc.dram_tensor("qt", [H, T], F32, kind="ExternalInput").ap()
    kt_d = nc.dram_tensor("kt", [H, T], F32, kind="ExternalInput").ap()
    v_d = nc.dram_tensor("v", [T, H], F32, kind="ExternalInput").ap()
    cos_d = nc.dram_tensor("costab", [HALF, T], F32, kind="ExternalInput").ap()
    sin_d = nc.dram_tensor("sintab", [HALF, T], F32, kind="ExternalInput").ap()
    bias_d = nc.dram_tensor("alibi", [128, NS], F32, kind="ExternalInput").ap()
    ot_d = nc.dram_tensor("ot", [H, T], F32, kind="ExternalOutput").ap()

    with tile.TileContext(nc) as tc, ExitStack() as ctx:
        const = ctx.enter_context(tc.tile_pool(name="const", bufs=1))
        rpool = ctx.enter_context(tc.tile_pool(name="ropeout", bufs=1))
        vpool = ctx.enter_context(tc.tile_pool(name="vpool", bufs=1))
        stage = ctx.enter_context(tc.tile_pool(name="stage", bufs=1))
        atp = ctx.enter_context(tc.tile_pool(name="atp", bufs=26))
        dn = ctx.enter_context(tc.tile_pool(name="dn", bufs=2))
        onp = ctx.enter_context(tc.tile_pool(name="onp", bufs=4))
        ps1p = ctx.enter_context(tc.tile_pool(name="ps1", bufs=3, space="PSUM"))
        ps2p = ctx.enter_context(tc.tile_pool(name="ps2", bufs=3, space="PSUM"))
        pdnp = ctx.enter_context(tc.tile_pool(name="pdn", bufs=2, space="PSUM"))

        # small constants: alibi bias (gpsimd queue), ones column for the
        # denominator partition-reduce matmuls, reciprocal magic row
        biasb = const.tile([128, NS], F32)
        nc.gpsimd.dma_start(biasb[:], bias_d[:])
        ones_f = const.tile([128, 1], F32)
        nc.vector.memset(ones_f[:], 1.0)
        ones_r = const.tile([128, 1], F32R)
        nc.vector.tensor_copy(ones_r[:], ones_f[:])
        magicb = const.tile([1, CHUNK], I32)
        nc.vector.memset(magicb[:], RECIP_MAGIC)

        # persistent fp32r operands for the two GEMMs
        qe = [rpool.tile([128, T], F32R, name=f"qe{i}", tag=f"qe{i}")
              for i in range(2)]
        ke = [rpool.tile([128, T], F32R, name=f"ke{i}", tag=f"ke{i}")
              for i in range(2)]
        vr = vpool.tile([128, NS * H], F32R)

        # full-width staging tiles, filled by per-chunk DMAs (subtile deps
        # let rope/GEMM1 start as soon as their columns land)
        cosb = stage.tile([128, T], F32, tag="cosb")
        sinb = stage.tile([128, T], F32, tag="sinb")
        ks0 = stage.tile([128, T], F32, tag="ks0")
        ks1 = stage.tile([128, T], F32, tag="ks1")
        qs0 = stage.tile([128, T], F32, tag="qs0")
        qs1 = stage.tile([128, T], F32, tag="qs1")

        def load_cols(cc):
            col = slice(cc * CHUNK, (cc + 1) * CHUNK)
            for dst, src in ((cosb, cos_d), (sinb, sin_d),
                             (ks0, kt_d[0:128, :]), (ks1, kt_d[128:256, :])):
                nc.sync.dma_start(dst[:, col], src[:, col])

        def load_q_cols(cc):
            col = slice(cc * CHUNK, (cc + 1) * CHUNK)
            nc.sync.dma_start(qs0[:, col], qt_d[0:128, col])
            nc.sync.dma_start(qs1[:, col], qt_d[128:256, col])

        def rope(src0, src1, dst, col, tmptag):
            """dst0[:,col] = s0*cos - s1*sin ; dst1[:,col] = s1*cos + s0*sin"""
            n = col.stop - col.start
            nc.vector.tensor_mul(dst[0][:, col], src0[:, col], cosb[:, col])
            tmp = stage.tile([128, n], F32, tag="rtmp", bufs=3,
                             name=f"tmp{tmptag}{col.start}")
            nc.vector.tensor_mul(tmp[:], src1[:, col], sinb[:, col])
            nc.vector.tensor_sub(dst[0][:, col], dst[0][:, col], tmp[:])
            nc.vector.tensor_mul(dst[1][:, col], src1[:, col], cosb[:, col])
            tmp2 = stage.tile([128, n], F32, tag="rtmp", bufs=3,
                              name=f"tmp2{tmptag}{col.start}")
            nc.vector.tensor_mul(tmp2[:], src0[:, col], sinb[:, col])
            nc.vector.tensor_add(dst[1][:, col], dst[1][:, col], tmp2[:])

        # chunk-0 inputs first, then k/q rope pipelined with remaining DMAs
        load_cols(0)
        load_q_cols(0)
        rope(ks0, ks1, ke, slice(0, CHUNK), "k0")
        rope(qs0, qs1, qe, slice(0, CHUNK), "q0")
        for cc in range(1, NCHUNK):
            load_cols(cc)
            load_q_cols(cc)
            rope(ks0, ks1, ke, slice(cc * CHUNK, (cc + 1) * CHUNK), f"k{cc}")

        # v load + fp32r cast entirely on gpsimd (own DMA queues, own ALU)
        for s in range(NS):
            vst = stage.tile([128, H], F32, tag="vst", bufs=4, name=f"vst{s}")
            nc.gpsimd.dma_start(vst[:], v_d[s * 128:(s + 1) * 128, :])
            nc.gpsimd.tensor_copy(vr[:, s * H:(s + 1) * H], vst[:])

        mm = nc.tensor.matmul
        for c in range(NCHUNK):
            tcol = slice(c * CHUNK, (c + 1) * CHUNK)
            if c + 1 < NCHUNK:
                # rope next chunk's q columns ahead of its GEMM1
                rope(qs0, qs1, qe, slice((c + 1) * CHUNK, (c + 2) * CHUNK),
                     f"q{c + 1}")
            at_tiles = []
            pden = pdnp.tile([1, CHUNK], F32)
            for s in range(NS):
                p1 = ps1p.tile([128, CHUNK], F32)
                mm(p1[:], ke[0][:, s * 128:(s + 1) * 128], qe[0][:, tcol],
                   start=True, stop=False)
                mm(p1[:], ke[1][:, s * 128:(s + 1) * 128], qe[1][:, tcol],
                   start=False, stop=True)
                if s > 0:
                    # denominator ones-matmul, one tile behind the exps so
                    # the PE never waits on the ACT stream
                    mm(pden[:], ones_r[:, 0:1], at_tiles[s - 1][:],
                       start=(s == 1), stop=False)
                at = atp.tile([128, CHUNK], F32R, tag="at")
                nc.scalar.activation(at[:], p1[:], EXP,
                                     bias=biasb[:, s:s + 1], scale=SCALE)
                at_tiles.append(at)
            mm(pden[:], ones_r[:, 0:1], at_tiles[NS - 1][:],
               start=False, stop=True)

            # reciprocal of the [1, CHUNK] denominator row:
            # seed r = bits(magic - bits(d)), then 3 Newton steps
            den_sb = dn.tile([1, CHUNK], F32, tag="den_sb")
            nc.vector.tensor_copy(den_sb[:], pden[0:1, :])
            r = dn.tile([1, CHUNK], F32, tag="rA", name=f"rA{c}")
            nc.vector.tensor_sub(r[:].bitcast(I32), magicb[:],
                                 den_sb[:].bitcast(I32))
            for it in range(2):
                t2 = dn.tile([1, CHUNK], F32, tag="nt", bufs=2,
                             name=f"nt{c}_{it}")
                nc.vector.scalar_tensor_tensor(t2[:], den_sb[:], -1.0, r[:],
                                               MULT, MULT)
                r_new = dn.tile([1, CHUNK], F32, tag=f"r{it % 2}", bufs=2,
                                name=f"r{c}_{it}")
                nc.vector.scalar_tensor_tensor(r_new[:], t2[:], 2.0, r[:],
                                               ADD, MULT)
                r = r_new
            recipb = dn.tile([128, CHUNK], F32, tag="recipb")
            nc.gpsimd.partition_broadcast(recipb[:], r[0:1, :], 128)

            for h in range(2):
                p2 = ps2p.tile([128, CHUNK], F32)
                for s in range(NS):
                    mm(p2[:], vr[:, s * H + h * 128: s * H + h * 128 + 128],
                       at_tiles[s][:], start=(s == 0), stop=(s == NS - 1))
                on = onp.tile([128, CHUNK], F32)
                nc.vector.tensor_mul(on[:], p2[:], recipb[:])
                nc.sync.dma_start(ot_d[h * 128:(h + 1) * 128, tcol], on[:])

    nc.compile()
    return nc


def _get_nc():
    if "nc" not in _NC_CACHE:
        _NC_CACHE["nc"] = _build_nc()
    return _NC_CACHE["nc"]


def _tables():
    j = np.arange(HALF, dtype=np.float64)
    inv = ROPE_BASE ** (-2.0 * j / H)
    t = np.arange(T, dtype=np.float64)
    fr = np.outer(inv, t)                       # [128, T]
    cos = np.cos(fr).astype(np.float32)
    sin = np.sin(fr).astype(np.float32)
    p = np.arange(128, dtype=np.float64)[:, None]
    sidx = p + 128.0 * np.arange(NS, dtype=np.float64)[None, :]
    bias = (SLOPE * sidx).astype(np.float32)    # [128, NS]
    return cos, sin, bias


def kernel(q, k, v):
    global LAST_RESULTS
    q = np.asarray(q, dtype=np.float32)
    k = np.asarray(k, dtype=np.float32)
    v = np.asarray(v, dtype=np.float32)
    assert q.shape == (B, T, H), q.shape

    nc = _get_nc()
    cos, sin, bias = _tables()
    in_maps = []
    for b in range(B):
        in_maps.append({
            "qt": np.ascontiguousarray(q[b].T),
            "kt": np.ascontiguousarray(k[b].T),
            "v": np.ascontiguousarray(v[b]),
            "costab": cos,
            "sintab": sin,
            "alibi": bias,
        })
    kw = {}
    if TRACE:
        kw = dict(trace=True)
    res = run_bass_kernel_spmd(nc, in_maps, list(range(B)), **kw)
    LAST_RESULTS = res
    out = np.stack(
        [np.ascontiguousarray(res.results[b]["ot"]).T for b in range(B)], axis=0
    )
    return out[None].astype(np.float32)
